# revision 7
# baseline (speedup 1.0000x reference)
"""Tensor-parallel MultiHeadAttention (QKV + RoPE + GQA causal SDPA + dense)
for 8 Trainium2 NeuronCores — bf16, kv-projection-deduplicated edition.

Sharding (TP as in TPMultiHeadAttention): core d owns query heads {2d, 2d+1}
and kv head d//2. NEW vs the baseline: the kv projection is not computed
redundantly on both cores of a pair — core 2g projects only k_g and core
2g+1 only v_g (128 rows instead of 256), and the pair exchanges raw kv
chunks through DRAM bounce buffers + a pairwise AllGather
(replica_groups=[[0,1],[2,3],[4,5],[6,7]]). This removes 1/4 of the QKV
matmul columns (~13.7us of PE streaming per core). The AllGather output is
flat-concat ordered [k_g, v_g], so every core reads both halves back
uniformly (SPMD-safe, no core-id branches). RoPE for k and the XBAR v
transposes are applied after the exchange, unchanged vs the baseline.

Per chunk: 16 units x 3 matmuls (f-order kv,q0,q1; kv psum finishes first)
-> kv psum->SBUF (scalar) -> DMA to cin -> AllGather -> k_raw/v_raw DMAd
back. The exchange latency (~8-10us) hides under later qkv chunks;
attention step (c, j) only needs kv chunk j//4, so attn(0) trickles in
chunk 2, attn(1) in chunk 3 + B0, attn(2)/(3) in phase B over the dense
filler. Rope arithmetic blobs are split per-head and placed between the
latency-critical mask/acc DVE ops (in-order queues).

Phase B: exp on ScalarE (1.12us/step) is the serializer; dense psum copies
are split vector/scalar per eo, and the softmax finalization of the LAST
chunk is per-128-column st-slice: denominator ones-matmul, reciprocal,
broadcast and ct multiply for slice st only depend on ctx/acc columns that
are final after ctx matmul j=12+st, so dense(3, st) starts while the
attention drain still runs. All matmul operands bf16; PSUM fp32. Host sums
the 8 bf16 partial outputs.
"""

import numpy as np
import ml_dtypes

B, S, E = 1, 2048, 2048
H, KVH, D = 16, 4, 128
NCORES = 8
P = 128
FD = 512            # matmul moving free dim == one fp32 PSUM bank
NE = E // P         # 16 contraction tiles over the embedding dim
NG = 4              # eo-groups of 4
NSC = S // FD       # 4 sequence chunks
NST = S // P        # 16 sequence tiles
FLOC = 3 * P        # local fused qkv rows per core (2 q heads + k XOR v)
ROPE_BASE = 10000.0
DIAG_START = (0, 128, 256, 384)
BF = ml_dtypes.bfloat16
GROUPS = [[0, 1], [2, 3], [4, 5], [6, 7]]

LAST_RESULT = None
_BASS_CACHE = None


def _rope_tables():
    inv = 1.0 / (ROPE_BASE ** (np.arange(0, D, 2, dtype=np.float64) / D))
    t = np.arange(S, dtype=np.float64)
    freqs = np.outer(t, inv)
    emb = np.concatenate([freqs, freqs], axis=-1)  # [S, D]
    return np.cos(emb), np.sin(emb)


def _host_constants():
    cos, sin = _rope_tables()
    cos_ds = np.ascontiguousarray(cos.T)  # [D, S]
    sin_ds = np.ascontiguousarray(sin.T)
    # sign-folded sin for the partition-shifted rotate-half:
    # tt[d] = qt[(d+64)%128] * sg[d],  sg = [-sin[:64]; +sin[64:]]
    sg = np.concatenate([-sin_ds[:64], sin_ds[64:]], axis=0)
    r_idx = np.arange(P)[:, None]
    c_idx = np.arange(P)[None, :]
    tri = (r_idx <= c_idx).astype(np.float64)
    return {
        "cosr": cos_ds.astype(BF),
        "sgsin": sg.astype(BF),
        "trim": tri.astype(BF),
        "ones": np.ones((P, 1), np.float64).astype(BF),
    }


def _build_bass():
    import concourse.mybir as mybir
    import concourse.tile as tile
    from concourse import bacc

    f32 = mybir.dt.float32
    bf16 = mybir.dt.bfloat16
    Exp = mybir.ActivationFunctionType.Exp

    nc = bacc.Bacc(None, target_bir_lowering=False, name="mha_tp8_v4")
    xG = nc.dram_tensor("xG", [NSC, NG, P, 4, FD], bf16, kind="ExternalInput")
    wG = nc.dram_tensor("wG", [NG, P, 4, FLOC], bf16, kind="ExternalInput")
    wdG = nc.dram_tensor("wdG", [P, 2, S], bf16, kind="ExternalInput")
    cosr = nc.dram_tensor("cosr", [P, S], bf16, kind="ExternalInput")
    sgsin = nc.dram_tensor("sgsin", [P, S], bf16, kind="ExternalInput")
    trim = nc.dram_tensor("trim", [P, P], bf16, kind="ExternalInput")
    ones = nc.dram_tensor("ones", [P, 1], bf16, kind="ExternalInput")
    out = nc.dram_tensor("out", [NSC, 4, P, 4, FD], bf16, kind="ExternalOutput")

    with tile.TileContext(nc) as tc:
        with tc.tile_pool(name="const", bufs=1) as const, \
             tc.tile_pool(name="ps_ctx", bufs=2, space="PSUM") as ps_ctx, \
             tc.tile_pool(name="xs_p", bufs=5) as xpool, \
             tc.tile_pool(name="rtmp", bufs=3) as rtmp, \
             tc.tile_pool(name="kv_p", bufs=2) as kvp, \
             tc.tile_pool(name="pt_p", bufs=8) as ptp, \
             tc.tile_pool(name="acc_p", bufs=2) as accp, \
             tc.tile_pool(name="dn_p", bufs=2) as dnp, \
             tc.tile_pool(name="ctx_p", bufs=3) as ctxp, \
             tc.tile_pool(name="out_p", bufs=3) as outp, \
             tc.tile_pool(name="dram", bufs=2, space="DRAM") as dram:
            w_sb = const.tile([P, NE, FLOC], bf16, name="w_sb")
            cq = const.tile([P, S], bf16, name="cq")
            sg = const.tile([P, S], bf16, name="sg")
            mk = const.tile([P, P], bf16, name="mk")
            wd_sb = const.tile([P, 2, S], bf16, name="wd_sb")
            qr = const.tile([P, 2, S], bf16, name="qr")
            kr = const.tile([P, S], bf16, name="kr")
            vT = const.tile([P, S], bf16, name="vT")
            vn = const.tile([P, NST, P], bf16, name="vn")
            on = const.tile([P, 1], bf16, name="on")
            warm = const.tile([P, 8], bf16, name="warm")

            # ---- shared attention machinery (paired heads per j-tile) ----
            st_ = {}          # per-chunk attention state
            all_csb = {}      # (c, h) -> [P, FD] tile, or (c, h, st) -> [P, P]

            def attn_begin(c):
                two = c >= 1
                ctxps = [
                    ps_ctx.tile([P, FD], f32, tag="ctx", name=f"ctx_{c}_{h}")
                    for h in range(2)
                ]
                accs = [
                    accp.tile([P, 2, FD], bf16, tag=f"acc{ch}", name=f"acc_{c}_{ch}")
                    for ch in range(2 if two else 1)
                ]
                st_[c] = (ctxps, accs, two)

            def attn_step(c, j, sdpool, split):
                ctxps, accs, two = st_[c]
                o = j - 4 * c
                so = DIAG_START[o] if o >= 0 else 0
                n = FD - so
                pt = ptp.tile([P, 2, FD], bf16, tag="pt", name=f"pt_{c}_{j}")
                if split:
                    # phase A: one psum bank per head so sd bufs=3 pipelines
                    for h in range(2):
                        sd = sdpool.tile([P, FD], f32, tag="sd",
                                         name=f"sd_{c}_{j}_{h}")
                        nc.tensor.matmul(
                            sd[:, :n],
                            lhsT=kr[:, j * P:(j + 1) * P],
                            rhs=qr[:, h, c * FD + so:(c + 1) * FD],
                            start=True, stop=True,
                        )
                        nc.scalar.activation(pt[:, h, :n], sd[:, :n], Exp)
                else:
                    sd = sdpool.tile([P, 2, FD], f32, tag="sd", name=f"sd_{c}_{j}")
                    for h in range(2):
                        nc.tensor.matmul(
                            sd[:, h, :n],
                            lhsT=kr[:, j * P:(j + 1) * P],
                            rhs=qr[:, h, c * FD + so:(c + 1) * FD],
                            start=True, stop=True,
                        )
                    nc.scalar.activation(pt[:, :, :n], sd[:, :, :n], Exp)
                if o >= 0:
                    for h in range(2):
                        nc.vector.tensor_mul(pt[:, h, :P], pt[:, h, :P], mk)
                acc = accs[j % 2 if two else 0]
                if j < (2 if two else 1):
                    nc.vector.tensor_copy(acc, pt)
                else:
                    nc.vector.tensor_add(acc[:, :, so:], acc[:, :, so:], pt[:, :, :n])
                return (j, pt, so, n)

            def attn_ctx(c, ent):
                ctxps, _, _ = st_[c]
                nj = 4 * c + 4
                j, pt, so, n = ent
                for h in range(2):
                    nc.tensor.matmul(
                        ctxps[h][:, so:],
                        lhsT=vn[:, j, :],
                        rhs=pt[:, h, :n],
                        start=(j == 0), stop=(j == nj - 1),
                    )

            def attn_tail(c, sdpool, ptag="sd"):
                # whole-chunk softmax finalize (chunks 0..2)
                ctxps, accs, two = st_[c]
                crs = []
                for h in range(2):
                    cr = ctxp.tile([P, FD], bf16, tag=f"cr{h}", name=f"cr_{c}_{h}")
                    nc.scalar.copy(cr, ctxps[h])
                    crs.append(cr)
                for h in range(2):
                    rp = sdpool.tile([1, FD], f32, tag=ptag, name=f"rp_{c}_{h}")
                    nc.tensor.matmul(rp, lhsT=on, rhs=accs[0][:, h, :],
                                     start=True, stop=not two)
                    if two:
                        nc.tensor.matmul(rp, lhsT=on, rhs=accs[1][:, h, :],
                                         start=False, stop=True)
                    rec = dnp.tile([1, FD], f32, tag=f"rec{h}", name=f"rec_{c}_{h}")
                    nc.vector.reciprocal_approx_fast(rec, rp)
                    rb = dnp.tile([P, FD], f32, tag=f"rb{h}", name=f"rb_{c}_{h}")
                    nc.gpsimd.partition_broadcast(rb, rec)
                    ct = ctxp.tile([P, FD], bf16, tag=f"ct{h}", name=f"csb_{c}_{h}")
                    nc.vector.tensor_mul(ct, crs[h], rb)
                    all_csb[(c, h)] = ct

            def fin_st(c, stt, rppool, ptag):
                # per-st-slice finalize for the last chunk: only needs
                # ctx/acc columns that are final after ctx matmul j=4c+stt
                ctxps, accs, two = st_[c]
                r = slice(stt * P, (stt + 1) * P)
                for h in range(2):
                    cr = ctxp.tile([P, P], bf16, tag=f"crs{h}",
                                   name=f"crs_{c}_{stt}_{h}")
                    nc.scalar.copy(cr, ctxps[h][:, r])
                    rp = rppool.tile([1, P], f32, tag=ptag,
                                     name=f"rps_{c}_{stt}_{h}")
                    nc.tensor.matmul(rp, lhsT=on, rhs=accs[0][:, h, r],
                                     start=True, stop=not two)
                    if two:
                        nc.tensor.matmul(rp, lhsT=on, rhs=accs[1][:, h, r],
                                         start=False, stop=True)
                    rec = dnp.tile([1, P], f32, tag=f"recs{h}",
                                   name=f"recs_{c}_{stt}_{h}")
                    nc.vector.reciprocal_approx_fast(rec, rp)
                    rb = dnp.tile([P, P], f32, tag=f"rbs{h}",
                                  name=f"rbs_{c}_{stt}_{h}")
                    nc.gpsimd.partition_broadcast(rb, rec)
                    ct = ctxp.tile([P, P], bf16, tag=f"cts{h}",
                                   name=f"cts_{c}_{stt}_{h}")
                    nc.vector.tensor_mul(ct, cr, rb)
                    all_csb[(c, h, stt)] = ct

            # ---- kv exchange: psum -> SBUF -> DRAM -> AllGather -> SBUF ----
            kv_parts = {}

            def kv_exchange(sc, kv_ps):
                ssl = slice(sc * FD, (sc + 1) * FD)
                kvsb = kvp.tile([P, FD], bf16, tag="kvsb", name=f"kvsb_{sc}")
                nc.scalar.copy(kvsb, kv_ps)
                cin = dram.tile([P, FD], bf16, tag="cin", name=f"cin_{sc}")
                cout = dram.tile([2, P, FD], bf16, tag="cout", name=f"cout_{sc}")
                # the whole exchange rides the gpsimd/SWDGE ring: a readback
                # on the sync ring would head-of-line block the next chunk's
                # x DMAs behind the collective (in-order engine queues)
                nc.gpsimd.dma_start(cin, kvsb)
                nc.gpsimd.collective_compute(
                    "AllGather",
                    mybir.AluOpType.bypass,
                    replica_groups=GROUPS,
                    ins=[cin.opt()],
                    outs=[cout.opt()],
                )
                kraw = kvp.tile([P, FD], bf16, tag="kraw", name=f"kraw_{sc}")
                nc.gpsimd.dma_start(kraw, cout[0])
                nc.gpsimd.dma_start(vT[:, ssl], cout[1])
                kv_parts[sc] = kraw

            def rope_arith(dst, src, ssl):
                ts = rtmp.tile([P, FD], bf16, tag="ts", name="ts")
                nc.vector.tensor_copy(ts[0:64, :], src[64:128, :])
                nc.vector.tensor_copy(ts[64:128, :], src[0:64, :])
                tt = rtmp.tile([P, FD], bf16, tag="tt", name="tt")
                nc.vector.tensor_mul(tt, ts, sg[:, ssl])
                nc.vector.tensor_mul(dst, src, cq[:, ssl])
                nc.vector.tensor_add(dst, dst, tt)

            qts = {}

            def rope_q_copy(sc, psums):
                qt0 = rtmp.tile([P, FD], bf16, tag="qt", name=f"qt_{sc}_0")
                nc.vector.tensor_copy(qt0, psums[0])
                qt1 = rtmp.tile([P, FD], bf16, tag="qt", name=f"qt_{sc}_1")
                nc.scalar.copy(qt1, psums[1])
                qts[sc] = (qt0, qt1)

            def rope_q_arith(sc, f):
                ssl = slice(sc * FD, (sc + 1) * FD)
                rope_arith(qr[:, f, ssl], qts[sc][f], ssl)

            def rope_k(sc):
                ssl = slice(sc * FD, (sc + 1) * FD)
                rope_arith(kr[:, ssl], kv_parts[sc], ssl)

            def vn_t(jt):
                nc.sync.dma_start_transpose(vn[:, jt, :], vT[:, jt * P:(jt + 1) * P])

            pend = {}
            pendB = []

            # ---- Phase A: QKV (dedup) + exchanges, attn(0/1) trickled ----
            with tc.tile_pool(name="ps_qkv", bufs=1, space="PSUM") as pqkv, \
                 tc.tile_pool(name="ps_sA", bufs=3, space="PSUM") as psA:
                # tables ride the idle gpsimd ring; wd rides the scalar ring
                # (idle after chunk 0's weight loads)
                nc.gpsimd.dma_start(mk, trim[:, :])
                nc.gpsimd.partition_broadcast(warm, mk[0:1, 0:8])
                nc.gpsimd.dma_start(cq, cosr[:, :])
                nc.gpsimd.dma_start(sg, sgsin[:, :])
                nc.gpsimd.dma_start(on, ones[:, :])

                qkv_ps = {}

                def qkv_units(sc):
                    """16 units; yields after each unit's 3 matmuls (~650ns)."""
                    psums = [
                        pqkv.tile([P, FD], f32, tag=f"qkv{f}", name=f"ps_qkv{f}_{sc}")
                        for f in range(3)
                    ]
                    qkv_ps[sc] = psums
                    for g in range(NG):
                        fine = sc == 0 and g == 0
                        if sc == 0 and not fine:
                            nc.scalar.dma_start(w_sb[:, 4 * g:4 * g + 4, :], wG[g])
                        xs = xpool.tile([P, 4, FD], bf16, tag="xs", name=f"xs_{sc}_{g}")
                        if not fine:
                            nc.sync.dma_start(xs, xG[sc, g])
                        for j in range(4):
                            if fine:
                                # 96KB w / 128KB x pieces so the first
                                # matmuls aren't starved
                                nc.scalar.dma_start(w_sb[:, j, :], wG[0, :, j, :])
                                nc.sync.dma_start(xs[:, j, :], xG[0, 0, :, j, :])
                            eo = 4 * g + j
                            # f-order (kv, q0, q1): kv psum completes first
                            for f in (2, 0, 1):
                                nc.tensor.matmul(
                                    psums[f],
                                    lhsT=w_sb[:, eo, f * P:(f + 1) * P],
                                    rhs=xs[:, j, :],
                                    start=(eo == 0),
                                    stop=(eo == NE - 1),
                                )
                            yield
                    if sc == 0:
                        nc.scalar.dma_start(wd_sb, wdG[:, :, :])

                def SA(c, j):
                    pend[(c, j)] = attn_step(c, j, psA, split=True)

                def XA(c, j):
                    attn_ctx(c, pend.pop((c, j)))

                # chunks 0, 1: plain (kv(0) exchange in flight during chunk 1)
                for _ in qkv_units(0):
                    pass
                kv_exchange(0, qkv_ps[0][2])
                rope_q_copy(0, qkv_ps[0])
                rope_q_arith(0, 0)
                rope_q_arith(0, 1)
                for _ in qkv_units(1):
                    pass
                kv_exchange(1, qkv_ps[1][2])
                rope_q_copy(1, qkv_ps[1])
                # chunk 2 + attn(0)
                attn_begin(0)
                sched2 = {
                    0: [lambda: rope_k(0)],
                    1: [lambda: vn_t(0)],
                    2: [lambda: rope_q_arith(1, 0)],
                    3: [lambda: vn_t(1)],
                    5: [lambda: vn_t(2)],
                    6: [lambda: SA(0, 0)],
                    8: [lambda: SA(0, 1), lambda: vn_t(3)],
                    9: [lambda: XA(0, 0)],
                    10: [lambda: SA(0, 2)],
                    11: [lambda: XA(0, 1), lambda: rope_q_arith(1, 1)],
                    12: [lambda: SA(0, 3)],
                    13: [lambda: XA(0, 2)],
                    15: [lambda: XA(0, 3)],
                }
                for i, _ in enumerate(qkv_units(2)):
                    for a in sched2.get(i, ()):
                        a()
                kv_exchange(2, qkv_ps[2][2])
                rope_q_copy(2, qkv_ps[2])
                # chunk 3 + attn(1) j0..4
                attn_begin(1)
                sched3 = {
                    0: [lambda: attn_tail(0, psA), lambda: rope_k(1)],
                    1: [lambda: SA(1, 0)],
                    2: [lambda: vn_t(4)],
                    4: [lambda: SA(1, 1)],
                    5: [lambda: XA(1, 0), lambda: rope_q_arith(2, 0)],
                    6: [lambda: vn_t(5)],
                    7: [lambda: SA(1, 2)],
                    8: [lambda: XA(1, 1)],
                    9: [lambda: vn_t(6)],
                    10: [lambda: SA(1, 3)],
                    11: [lambda: XA(1, 2), lambda: rope_q_arith(2, 1)],
                    12: [lambda: vn_t(7)],
                    13: [lambda: SA(1, 4)],
                    14: [lambda: XA(1, 3)],
                }
                for i, _ in enumerate(qkv_units(3)):
                    for a in sched3.get(i, ()):
                        a()
                kv_exchange(3, qkv_ps[3][2])
                rope_q_copy(3, qkv_ps[3])

            # ---- Phase B: attn(1) drain, attn(2/3) + dense ----
            def make_dense_units(pool, engines):
                def csb(c, h, stt):
                    if (c, h, stt) in all_csb:
                        return all_csb[(c, h, stt)]
                    return all_csb[(c, h)][:, stt * P:(stt + 1) * P]

                def dense_st(c, stt, tail):
                    ot = outp.tile([P, 4, FD], bf16, tag="ot", name=f"ot_{c}_{stt}")
                    for eo in range(4):
                        op = pool.tile([P, FD], f32, tag="o",
                                       name=f"o_{c}_{stt}_{eo}")
                        for h in range(2):
                            nc.tensor.matmul(
                                op,
                                lhsT=csb(c, h, stt),
                                rhs=wd_sb[:, h, eo * FD:(eo + 1) * FD],
                                start=(h == 0), stop=(h == 1),
                            )
                        if engines[eo] == "s":
                            nc.scalar.copy(ot[:, eo, :], op)
                        else:
                            nc.vector.tensor_copy(ot[:, eo, :], op)
                        if tail:
                            nc.sync.dma_start(out[c, stt, :, eo, :], ot[:, eo, :])
                        elif eo == 3:
                            nc.sync.dma_start(out[c, stt], ot)
                        yield

                def dense_units(c):
                    for stt in range(4):
                        yield from dense_st(c, stt, False)
                return dense_units, dense_st

            with tc.tile_pool(name="ps_sB", bufs=2, space="PSUM") as psB, \
                 tc.tile_pool(name="ps_o", bufs=2, space="PSUM") as ps_o:
                dense_units, dense_st = make_dense_units(ps_o, ["v", "v", "v", "s"])

                def SB(c, j):
                    pendB.append((c, attn_step(c, j, psB, split=False)))

                def XB(c):
                    cc, ent = pendB.pop(0)
                    assert cc == c
                    attn_ctx(c, ent)

                # B0: attn(1) j5..7 drain with dense(0) starting
                dq0 = dense_units(0)
                XA(1, 4)
                SB(1, 5)
                next(dq0); next(dq0)
                SB(1, 6)
                XB(1)
                next(dq0)
                rope_q_arith(3, 0)
                SB(1, 7)
                XB(1)
                next(dq0); next(dq0)
                XB(1)
                next(dq0)
                attn_tail(1, psB)
                next(dq0); next(dq0)

                # B1: attn(2) j0..11 with rest of dense(0)
                attn_begin(2)
                extras1 = {
                    0: [lambda: rope_k(2), lambda: vn_t(8)],
                    2: [lambda: vn_t(9)],
                    3: [lambda: rope_q_arith(3, 1)],
                    5: [lambda: vn_t(10)],
                    7: [lambda: vn_t(11)],
                }
                nd = 8
                for j in range(12):
                    for a in extras1.get(j, ()):
                        a()
                    SB(2, j)
                    want = 8 + (j + 1) * 8 // 12
                    while nd < want:
                        next(dq0)
                        nd += 1
                    if len(pendB) >= 3:
                        XB(2)
                while pendB:
                    XB(2)
                for _ in dq0:
                    pass
                attn_tail(2, psB)

                # B2: attn(3) j0..11 with dense(1)
                attn_begin(3)
                dq1 = dense_units(1)
                extras2 = {
                    0: [lambda: rope_k(3)],
                    1: [lambda: vn_t(12)],
                    3: [lambda: vn_t(13)],
                    5: [lambda: vn_t(14)],
                    7: [lambda: vn_t(15)],
                }
                nd = 0
                for j in range(12):
                    for a in extras2.get(j, ()):
                        a()
                    SB(3, j)
                    want = (j + 1) * 16 // 12
                    while nd < want:
                        next(dq1)
                        nd += 1
                    if len(pendB) >= 3:
                        XB(3)
                # pendB now holds j9, j10, j11

                # B3: attn(3) j12..15 drain; per-st finalize; dense(2)+(3)
                dense_units2, dense_st2 = make_dense_units(ps_o, ["v", "s", "v", "s"])
                dq2 = dense_units2(2)

                def meter(k):
                    for _ in range(k):
                        next(dq2)

                SB(3, 12); XB(3); meter(2)           # ctx j10
                SB(3, 13); XB(3); meter(2)           # ctx j11
                SB(3, 14); XB(3); meter(2)           # ctx j12
                fin_st(3, 0, ps_o, "o")
                SB(3, 15); XB(3); meter(2)           # ctx j13
                fin_st(3, 1, ps_o, "o")
                for i, _ in enumerate(dense_st(3, 0, False)):
                    if i == 1:
                        meter(1)
                XB(3); meter(1)                      # ctx j14
                fin_st(3, 2, ps_o, "o")
                for i, _ in enumerate(dense_st(3, 1, False)):
                    if i == 1:
                        meter(1)
                XB(3)                                # ctx j15 (stop)
                fin_st(3, 3, ps_o, "o")
                for i, _ in enumerate(dense_st(3, 2, False)):
                    if i == 1:
                        meter(2)
                for _ in dq2:
                    pass
                for _ in dense_st2(3, 3, True):
                    pass
    nc.compile()
    return nc


def make_in_maps(x, w_qkv, w_dense):
    x = np.asarray(x, np.float32).reshape(S, E)
    w_qkv = np.asarray(w_qkv, np.float32)
    w_dense = np.asarray(w_dense, np.float32)
    # x^T tiled to [sc, g, p, j, f] so each 512KB DMA block is contiguous
    xT = np.ascontiguousarray(x.T)
    xG = np.ascontiguousarray(
        xT.reshape(NG, 4, P, NSC, FD).transpose(3, 0, 2, 1, 4)
    ).astype(BF)
    consts = _host_constants()
    in_maps = []
    scale = np.float64(1.0 / np.sqrt(D))
    for d in range(NCORES):
        g = d // 2
        wq = w_qkv[2 * d * P:(2 * d + 2) * P] * scale
        if d % 2 == 0:
            wkv = w_qkv[H * D + g * P: H * D + (g + 1) * P]          # k head g
        else:
            wkv = w_qkv[H * D + KVH * D + g * P:
                        H * D + KVH * D + (g + 1) * P]               # v head g
        # f-blocks: [q0, q1, kv]
        wqkvT_d = np.ascontiguousarray(np.concatenate([wq, wkv], 0).T)
        wG_d = np.ascontiguousarray(
            wqkvT_d.reshape(NG, 4, P, FLOC).transpose(0, 2, 1, 3)
        ).astype(BF)
        wdT_d = w_dense[:, 2 * d * P:(2 * d + 2) * P].T  # [2P, S]
        wdG_d = np.ascontiguousarray(
            wdT_d.reshape(2, P, S).transpose(1, 0, 2)
        ).astype(BF)
        m = {"xG": xG, "wG": wG_d, "wdG": wdG_d}
        m.update(consts)
        in_maps.append(m)
    return in_maps


def kernel(x, w_qkv, w_dense):
    global LAST_RESULT, _BASS_CACHE
    from concourse.bass_utils import run_bass_kernel_spmd

    in_maps = make_in_maps(x, w_qkv, w_dense)
    if _BASS_CACHE is None:
        _BASS_CACHE = _build_bass()
    res = run_bass_kernel_spmd(_BASS_CACHE, in_maps, core_ids=list(range(NCORES)))
    LAST_RESULT = res
    # sum partials over cores; [c, st, p, eo, f] flattens straight to [s, e]
    acc = np.zeros((NSC, 4, P, 4, FD), np.float32)
    for r in res.results:
        acc += r["out"].astype(np.float32)
    return np.ascontiguousarray(acc.reshape(S, E)).reshape(B, S, E)
